# revision 71
# baseline (speedup 1.0000x reference)
"""nn_AttentionBlock Trainium2 Bass kernel.

kernel(**inputs): FULL unsharded inputs (x [8,512,32,32], gamma/beta [512],
w_qkv [1536,512], b_qkv [1536], w_proj [512,512], b_proj [512]) -> FULL
output [8,512,32,32] float32.

Sharding: data-parallel over batch, one batch element per NeuronCore (8
cores), no collectives. Per core:
  GroupNorm: per-channel (sum, sumsq) stats (DVE reduce + ACT Square-accum;
  Square shares the exp ACT table set so no table reloads), group-combined
  with tiny mask matmuls; rstd = 1/sqrt(var+eps) via DVE bit-trick seed +
  2 Newton steps (no Ln -> single ACT table set).
  xn stored fp8e4; q/k/v projections and out-proj run as fp8e4 DoubleRow
  matmuls (2 contraction rows/cell). k-bias dropped (softmax-invariant),
  q-bias fused into the PSUM->SBUF copy. q/k stored bf16 (enables FWL on
  the per-tile score weight loads). Scores per head-pair use disjoint
  PE row-groups (tile_position (0,0)/(64,0)). exp on ACT with a -2
  logit shift (cancels in softmax) to stay under fp8e4 max; P stored fp8
  pair-interleaved for the DoubleRow AV. Softmax denominator via an extra
  ones-column of the AV lhsT; 1/r by int bit-trick (magic - i, with a 2^-6
  scale folded in for the fp8 A tiles), broadcast SBUF->SBUF with a
  stride-0 free-dim DMA. proj accumulates 64*Wp@(A/64) + I@(x+b_eff) in
  PSUM (b_proj and W_proj@b_v folded host-side into b_eff).
  Hardware loop unrolled 32x with ping-pong x prefetch; each body hoists
  the next iteration's GroupNorm over its proj tail (within a trip only --
  engine deps cannot cross the For_i back-edge semaphore reset).
"""
import sys, os
sys.path.insert(0, '/opt/trn_rl_repo')
import numpy as np

BCAST_MODE = os.environ.get("K_BCAST", "sbuf")   # sbuf | dram
UNROLL_ENV = int(os.environ.get("K_UNROLL", "32"))
EXPOFF = int(os.environ.get("K_EXPOFF", "0"))  # trailing t-tiles per head exp'd on DVE
AVDR = os.environ.get("K_AVDR", "1") == "1"    # fp8e4 DoubleRow AV matmuls
FP8ALL = os.environ.get("K_FP8", "1") == "1"   # fp8e4 DoubleRow qkv/vt/proj
import concourse.bacc as bacc
import concourse.mybir as mybir
import concourse.tile as tile

F32 = mybir.dt.float32
F32R = mybir.dt.float32r
BF16 = mybir.dt.bfloat16
AF = mybir.ActivationFunctionType
ALU = mybir.AluOpType
FP8 = mybir.dt.float8e4

C, S, NH, HD, G = 512, 1024, 8, 64, 32
KS = 4          # k-subtiles of 128 over C
SC = 2          # s-chunks of 512
TC = 8          # t-chunks of 128
OC = 4          # output channel chunks of 128
NP = 4          # head pairs
EPS = 1e-5


def emit_x_load(nc, pools, dram):
    x_sb = pools["xp"].tile([128, OC, S], F32R, tag="x")
    xr = dram["x"].ap().rearrange("(j p) s -> p j s", p=128)
    for j in range(OC):
        for half in range(2):
            nc.sync.dma_start(x_sb[:, j, half * 512:(half + 1) * 512],
                              xr[:, j, half * 512:(half + 1) * 512])
    return x_sb


def emit_gn_sums(nc, pools, x_sb):
    """GroupNorm part 1: per-channel sums on DVE (cheap to hide mid-body)."""
    sb1 = pools["sb1"]
    stats2 = sb1.tile([128, OC, 2], F32, tag="stats2")
    for j in range(OC):
        nc.vector.reduce_sum(out=stats2[:, j, 0:1], in_=x_sb[:, j, :],
                             axis=mybir.AxisListType.X)
    return stats2


def emit_gn(nc, pools, p_const, x_sb, stats2=None):
    """GroupNorm part 2: sumsq (ACT Square-accum; Square is in the exp
    table set -> no table reload) -> group combine -> rstd -> xn."""
    sb1 = pools["sb1"]
    ps_sm = pools["ps_av"]
    if stats2 is None:
        stats2 = emit_gn_sums(nc, pools, x_sb)
    sq_scr = sb1.tile([128, 2, S], F32, tag="sq_scr")
    for j in range(OC):
        nc.scalar.activation(out=sq_scr[:, j % 2, :], in_=x_sb[:, j, :],
                             func=AF.Square, accum_out=stats2[:, j, 1:2])

    # group combine: gstats[g, st] = sum_{c in g} stats2[c, st] / 16
    gs_ps = ps_sm.tile([32, 2], F32, tag="ps_av")
    for j in range(OC):
        nc.tensor.matmul(gs_ps[:], p_const["gn_fwd"][:, j, :], stats2[:, j, :],
                         start=(j == 0), stop=(j == OC - 1))
    gs_sb = sb1.tile([32, 2], F32, tag="gs_sb")
    nc.vector.tensor_copy(gs_sb[:], gs_ps[:])
    gm = sb1.tile([32, 2], F32, tag="gm")          # (M_g, rstd_g)
    gv = sb1.tile([32, 1], F32, tag="gv")
    INV = 1.0 / 1024.0
    nc.vector.tensor_scalar_mul(gm[:, 0:1], gs_sb[:, 0:1], INV)
    nc.vector.tensor_scalar_mul(gv[:], gs_sb[:, 1:2], INV)
    msq = sb1.tile([32, 1], F32, tag="msq")
    nc.vector.tensor_mul(msq[:], gm[:, 0:1], gm[:, 0:1])
    nc.vector.tensor_tensor(gv[:], gv[:], msq[:], ALU.subtract)
    nc.vector.tensor_scalar_add(gv[:], gv[:], EPS)
    # rstd = 1/sqrt(var+eps) on DVE: bit-trick seed + one Newton step
    # (keeps ACT on the exp-only table set -> no per-iteration table loads)
    rs_i = sb1.tile([32, 1], mybir.dt.int32, tag="rs_i")
    nc.vector.tensor_scalar(out=rs_i[:], in0=gv[:].bitcast(mybir.dt.int32),
                            scalar1=1, scalar2=None, op0=ALU.arith_shift_right)
    nc.vector.tensor_scalar(out=rs_i[:], in0=rs_i[:], scalar1=-1,
                            scalar2=0x5f3759df, op0=ALU.mult, op1=ALU.add)
    y0 = rs_i[:].bitcast(F32)
    t2 = sb1.tile([32, 1], F32, tag="t2")
    nc.vector.tensor_mul(t2[:], y0, y0)
    nc.vector.tensor_mul(t2[:], t2[:], gv[:])
    nc.vector.tensor_scalar(out=t2[:], in0=t2[:], scalar1=-0.5, scalar2=1.5,
                            op0=ALU.mult, op1=ALU.add)
    nc.vector.tensor_mul(gm[:, 1:2], y0, t2[:])
    # second Newton step for accuracy margin
    nc.vector.tensor_mul(t2[:], gm[:, 1:2], gm[:, 1:2])
    nc.vector.tensor_mul(t2[:], t2[:], gv[:])
    nc.vector.tensor_scalar(out=t2[:], in0=t2[:], scalar1=-0.5, scalar2=1.5,
                            op0=ALU.mult, op1=ALU.add)
    nc.vector.tensor_mul(gm[:, 1:2], gm[:, 1:2], t2[:])

    # broadcast back per channel chunk: means to cols [0:OC], rstds to [OC:2OC]
    mb_ps = ps_sm.tile([128, 2 * OC], F32, tag="ps_av")
    for j in range(OC):
        nc.tensor.matmul(mb_ps[:, j:j + 1], p_const["gn_bwd"][:, j, :], gm[:, 0:1],
                         start=True, stop=True)
        nc.tensor.matmul(mb_ps[:, OC + j:OC + j + 1], p_const["gn_bwd"][:, j, :], gm[:, 1:2],
                         start=True, stop=True)
    Acol = sb1.tile([128, OC], F32, tag="Acol")
    Bcol = sb1.tile([128, OC], F32, tag="Bcol")
    nc.vector.tensor_mul(Acol[:], mb_ps[:, OC:2 * OC], p_const["gamma"][:])
    nc.vector.tensor_mul(Bcol[:], mb_ps[:, 0:OC], Acol[:])
    nc.vector.tensor_tensor(Bcol[:], p_const["beta"][:], Bcol[:], ALU.subtract)
    xn_sb = sb1.tile([128, KS, S], FP8 if FP8ALL else F32R, tag="xn")
    for j in range(OC):
        if j < 2:
            nc.scalar.activation(out=xn_sb[:, j, :], in_=x_sb[:, j, :], func=AF.Identity,
                                 bias=Bcol[:, j:j + 1], scale=Acol[:, j:j + 1])
        else:
            nc.vector.tensor_scalar(out=xn_sb[:, j, :], in0=x_sb[:, j, :],
                                    scalar1=Acol[:, j:j + 1], scalar2=Bcol[:, j:j + 1],
                                    op0=ALU.mult, op1=ALU.add)

    return xn_sb


def emit_body(nc, tc_ctx, pools, dram, p_const, x_sb, xn_sb, prefetch=False,
              hoist_gn=False, pre=None):
    y_d = dram["y"]
    sb1 = pools["sb1"]
    qkp = pools["qk"]
    exp_p = pools["exp"]
    ap_ = pools["a"]
    rp = pools["recip"]
    ps_st = pools["ps_st"]     # score tiles only: ping-pong at ACT pace
    ps_av = pools["ps_av"]     # qkv/av/proj/gn psum
    x_next = emit_x_load(nc, pools, dram) if prefetch else None
    if xn_sb is None:
        xn_sb = emit_gn(nc, pools, p_const, x_sb)

    # ---------------- emit helpers ----------------
    qk_sb = {}

    def emit_qkv_chunk(h, xn=None):
        # chunk 2p: q rows for head pair p; chunk 2p+1: k rows.
        # k-bias is dropped (softmax-invariant); q-bias fused into the copy.
        xn = xn_sb if xn is None else xn
        QKDT = BF16 if os.environ.get("K_QKBF16", "1") == "1" else F32R
        qk_t = qkp.tile([128, S], QKDT, tag="qk", name=f"qk{h}")
        for sc in range(SC):
            qkv_ps = ps_av.tile([128, 512], F32, tag="ps_av", name=f"qkvps{h}_{sc}")
            ssl = slice(sc * 512, (sc + 1) * 512)
            if FP8ALL:
                for u in range(KS // 2):
                    nc.tensor.matmul(qkv_ps[:],
                                     p_const["wqk"][:, 2 * u:2 * u + 2, h, :],
                                     xn[:, 2 * u:2 * u + 2, ssl],
                                     start=(u == 0), stop=(u == KS // 2 - 1),
                                     perf_mode=mybir.MatmulPerfMode.DoubleRow)
            else:
                for ks in range(KS):
                    nc.tensor.matmul(qkv_ps[:],
                                     p_const["wqk"][:, ks, h, :],
                                     xn[:, ks, ssl],
                                     start=(ks == 0), stop=(ks == KS - 1))
            if h % 2 == 0:
                nc.vector.tensor_scalar_add(qk_t[:, ssl], qkv_ps[:],
                                            p_const["bqk"][:, h:h + 1])
            else:
                nc.vector.tensor_copy(qk_t[:, ssl], qkv_ps[:])
        qk_sb[h] = qk_t
        return qk_t

    av_lhs = p_const["av_lhs"]

    def emit_vt():
        for t in range(TC):
            vt_ps = ps_av.tile([128, 512], F32, tag="ps_av", name=f"vtps{t}")
            tsl = slice(t * 128, (t + 1) * 128)
            if FP8ALL:
                for u in range(KS // 2):
                    nc.tensor.matmul(vt_ps[:], xn_sb[:, 2 * u:2 * u + 2, tsl],
                                     p_const["wvt"][:, 2 * u:2 * u + 2, :],
                                     start=(u == 0), stop=(u == KS // 2 - 1),
                                     perf_mode=mybir.MatmulPerfMode.DoubleRow)
            else:
                for ks in range(KS):
                    nc.tensor.matmul(vt_ps[:], xn_sb[:, ks, tsl],
                                     p_const["wvt"][:, ks, :],
                                     start=(ks == 0), stop=(ks == KS - 1))
            # strided copies pack v for all 8 heads of this t-block
            if AVDR:
                dst = av_lhs[:, t // 2, :, t % 2, :].rearrange(
                    "p (pr tw) m -> p pr tw m", tw=2)
            else:
                dst = av_lhs[:, t].rearrange("p (pr tw) m -> p pr tw m", tw=2)
            srcv = vt_ps[:].rearrange("p (pr tw m) -> p pr tw m", tw=2, m=64)
            nc.vector.tensor_copy(dst[:, :, 0, 0:64], srcv[:, :, 0, :])
            nc.vector.tensor_copy(dst[:, :, 1, 64:128], srcv[:, :, 1, :])

    if FP8ALL:
        # A scaled by 1/64 (folded into the reciprocal magic); proj weights
        # are host-scaled by 64. Pair-grouped [128, 2, S] for DoubleRow.
        A_pair = [ap_.tile([128, 2, S], FP8, tag="A", name=f"Ap{u}") for u in range(NP // 2)]
    else:
        A_sb = [ap_.tile([128, S], F32R, tag="A", name=f"A{j}") for j in range(NP)]

    def emit_scores_pair(p, qq=None, kk=None):
        """Scores+exp for heads 2p (rows 0:64) and 2p+1 (rows 64:128) on
        disjoint PE row-groups (tile_position auto (0,0)/(64,0))."""
        qq = qk_sb[2 * p] if qq is None else qq
        kk = qk_sb[2 * p + 1] if kk is None else kk
        lo_e, lo_o = slice(0, 64), slice(64, 128)
        et_e, et_o = [], []
        for t in range(TC):
            tsl = slice(t * 128, (t + 1) * 128)
            st_e = ps_st.tile([128, S], F32, tag="ps_st", name=f"ste{p}_{t}")
            st_o = ps_st.tile([128, S], F32, tag="ps_st", name=f"sto{p}_{t}")
            # grouped per head so each lhsT is loaded once per t (ldw-opt is
            # off in this toolchain); the odd-head MMs still overlap the
            # even head's tail via the disjoint row-groups.
            for sc in range(SC):
                ssl = slice(sc * 512, (sc + 1) * 512)
                nc.tensor.matmul(st_e[:, ssl], kk[lo_e, tsl], qq[lo_e, ssl],
                                 start=True, stop=True)
            for sc in range(SC):
                ssl = slice(sc * 512, (sc + 1) * 512)
                nc.tensor.matmul(st_o[:, ssl], kk[lo_o, tsl], qq[lo_o, ssl],
                                 start=True, stop=True)
            if AVDR:
                if t % 2 == 0:
                    e0 = exp_p.tile([128, 2, S], FP8, tag="expst", name=f"e{2*p}_{t//2}")
                    e1 = exp_p.tile([128, 2, S], FP8, tag="expst", name=f"e{2*p+1}_{t//2}")
                    et_e.append(e0)
                    et_o.append(e1)
                for st, e in ((st_e, et_e[-1]), (st_o, et_o[-1])):
                    # -2 logit shift keeps exp under fp8e4 max (448); it
                    # scales P and r identically so A = av/r is unchanged.
                    nc.scalar.activation(out=e[:, t % 2, :], in_=st[:],
                                         func=AF.Exp, scale=0.125,
                                         bias=p_const["negc"][:])
            else:
                e0 = exp_p.tile([128, S], BF16, tag="expst", name=f"e{2*p}_{t}")
                e1 = exp_p.tile([128, S], BF16, tag="expst", name=f"e{2*p+1}_{t}")
                for st, e in ((st_e, e0), (st_o, e1)):
                    if t >= TC - EXPOFF:
                        # Schraudolph exp on DVE: bf16 bits via int16 affine
                        # of the logit; offloads ACT (the mid-phase pacer).
                        nc.vector.tensor_scalar(
                            out=e[:].bitcast(mybir.dt.int16), in0=st[:],
                            scalar1=0.125 * 184.6646742, scalar2=16249.0,
                            op0=ALU.mult, op1=ALU.add)
                    else:
                        nc.scalar.activation(out=e[:], in_=st[:], func=AF.Exp, scale=0.125)
                et_e.append(e0)
                et_o.append(e1)
        return et_e, et_o

    rsp = pools["rs"]

    def emit_av(h, exp_tiles, split=False):
        """AV for head h; r (denominator) lands on partition 64 (even) /
        0 (odd) via the ones-column of av_lhs. 1/r is extracted straight
        from PSUM (fused reciprocal) and staged to DRAM for broadcast.
        With split=True the r-path runs per 512-col half so the first
        half's divide overlaps the second half's matmuls (tail latency)."""
        import concourse.bass as bass
        av_ps = ps_st.tile([128, S], F32, tag="ps_st", name=f"av{h}")
        r_row = slice(64, 65) if h % 2 == 0 else slice(0, 1)
        p, half = h // 2, h % 2
        a_sl = slice(0, 64) if half == 0 else slice(64, 128)
        rb = rp.tile([128, S], F32, tag="rb", name=f"rb{h}")

        def rpath(ssl):
            n = ssl.stop - ssl.start
            rt = rsp.tile([1, S], F32, tag="rs", name=f"rs{h}_{ssl.start}")
            # 1/r via int bit-trick (magic - i): reciprocal_approx_fast
            # reads PSUM wrongly on HW; plain tensor_scalar is safe. Seed
            # error (<=5%) scales A, which proj attenuates by ~0.02.
            magic = 0x7EF311C3 - (6 << 23) if FP8ALL else 0x7EF311C3
            nc.vector.tensor_scalar(out=rt[0:1, 0:n].bitcast(mybir.dt.int32),
                                    in0=av_ps[r_row, ssl].bitcast(mybir.dt.int32),
                                    scalar1=-1, scalar2=magic,
                                    op0=ALU.mult, op1=ALU.add)
            eng = nc.gpsimd if h % 2 == 0 else nc.sync
            if BCAST_MODE == "sbuf":
                # SBUF->SBUF broadcast: re-read the single staged row 64x
                rt_ap = rt[0:1, 0:n]
                bcast_src = bass.AP(tensor=rt_ap.tensor, offset=rt_ap.offset,
                                    ap=[[1, 1], [0, 64], [1, n]])
                eng.dma_start(rb[a_sl, ssl], bcast_src)
            else:
                nc.sync.dma_start(dram["r_scr"].ap()[h:h + 1, ssl],
                                  rt[0:1, 0:n])
                bcast_src = bass.AP(tensor=dram["r_scr"], offset=h * S + ssl.start,
                                    ap=[[0, 64], [1, n]])
                eng.dma_start(rb[a_sl, ssl], bcast_src)
            if FP8ALL:
                nc.vector.tensor_mul(A_pair[p // 2][a_sl, p % 2, ssl],
                                     av_ps[a_sl, ssl], rb[a_sl, ssl])
            else:
                nc.vector.tensor_mul(A_sb[p][a_sl, ssl], av_ps[a_sl, ssl], rb[a_sl, ssl])

        for sc in range(SC):
            ssl = slice(sc * 512, (sc + 1) * 512)
            if AVDR:
                for u in range(TC // 2):
                    nc.tensor.matmul(av_ps[:, ssl], av_lhs[:, u, h, :, :],
                                     exp_tiles[u][:, :, ssl],
                                     start=(u == 0), stop=(u == TC // 2 - 1),
                                     perf_mode=mybir.MatmulPerfMode.DoubleRow)
            else:
                for t in range(TC):
                    nc.tensor.matmul(av_ps[:, ssl], av_lhs[:, t, h, :],
                                     exp_tiles[t][:, ssl],
                                     start=(t == 0), stop=(t == TC - 1))
            if split:
                rpath(ssl)
        if not split:
            rpath(slice(0, S))
        return av_ps

    def emit_beff():
        # b_eff fold now that raw x is consumed
        for j in range(OC):
            nc.vector.tensor_scalar_add(x_sb[:, j, :], x_sb[:, j, :], p_const["b_eff"][:, j:j + 1])

    def emit_proj():
        # psum = sum_ks Wp[ks].T @ A[ks] + I @ (x+b_eff); evacuation copies
        # alternate ACT/DVE (both idle at the tail), then y DMA from SBUF.
        # proj shares the ps_av slots (free by the tail).
        yr = y_d.ap().rearrange("(j p) s -> p j s", p=128)
        outp = pools["out"]
        for oc in range(OC):
            pj_ps = ps_st.tile([128, S], F32, tag="ps_st", name=f"pj{oc}")
            osl = slice(oc * 128, (oc + 1) * 128)
            for sc in range(SC):
                ssl = slice(sc * 512, (sc + 1) * 512)
                if FP8ALL:
                    for u in range(KS // 2):
                        nc.tensor.matmul(pj_ps[:, ssl],
                                         p_const["wpt"][:, 2 * u:2 * u + 2, osl],
                                         A_pair[u][:, :, ssl],
                                         start=(u == 0), stop=False,
                                         perf_mode=mybir.MatmulPerfMode.DoubleRow)
                else:
                    for ks in range(KS):
                        nc.tensor.matmul(pj_ps[:, ssl],
                                         p_const["wpt"][:, ks, osl],
                                         A_sb[ks][:, ssl],
                                         start=(ks == 0), stop=False)
                nc.tensor.matmul(pj_ps[:, ssl], p_const["ident"][:],
                                 x_sb[:, oc, ssl], start=False, stop=True)
            o_t = outp.tile([128, S], F32, tag="o", name=f"o{oc}")
            nc.vector.tensor_copy(o_t[:], pj_ps[:])
            nc.sync.dma_start(yr[:, oc, :], o_t[:])

    # ---------------- pipelined schedule ----------------
    if pre is None:
        emit_qkv_chunk(0)
        emit_qkv_chunk(1)
        et0, et1 = emit_scores_pair(0)
    else:
        qk_sb[0], qk_sb[1], et0, et1 = pre
        if et0 is None:
            et0, et1 = emit_scores_pair(0)
    emit_qkv_chunk(2)
    emit_qkv_chunk(3)
    emit_vt()
    emit_av(0, et0)
    emit_av(1, et1)
    et2, et3 = emit_scores_pair(1)
    emit_qkv_chunk(4)
    emit_qkv_chunk(5)
    emit_av(2, et2)
    emit_av(3, et3)
    stats_next = emit_gn_sums(nc, pools, x_next) if hoist_gn else None
    et4, et5 = emit_scores_pair(2)
    emit_qkv_chunk(6)
    emit_qkv_chunk(7)
    emit_beff()
    emit_av(4, et4)
    emit_av(5, et5)
    et6, et7 = emit_scores_pair(3)
    # Software-pipelined lead-in for the NEXT iteration (within a trip only:
    # engine deps may not cross the For_i back edge): GroupNorm and qkv
    # chunks 0/1 before av6/av7 so their DVE copies precede the tail's
    # reciprocal/multiply chain in the DVE queue; scores pair 0 after proj
    # (st allocations behind proj's avoids the PRE=2 slot deadlock).
    xn_next = emit_gn(nc, pools, p_const, x_next, stats_next) if hoist_gn else None
    PRE_MODE = int(os.environ.get("K_PRE", "0"))  # 0=none 1=qkv 2=qkv+scores-pre 3=qkv+scores-post
    pre_next = None
    if hoist_gn and PRE_MODE > 0:
        q0 = emit_qkv_chunk(0, xn=xn_next)
        q1 = emit_qkv_chunk(1, xn=xn_next)
        if PRE_MODE == 2:
            e0n, e1n = emit_scores_pair(0, qq=q0, kk=q1)
            pre_next = (q0, q1, e0n, e1n)
        else:
            pre_next = (q0, q1, None, None)
    emit_av(6, et6, split=True)
    emit_av(7, et7, split=True)
    emit_proj()
    if hoist_gn and PRE_MODE == 3:
        e0n, e1n = emit_scores_pair(0, qq=pre_next[0], kk=pre_next[1])
        pre_next = (pre_next[0], pre_next[1], e0n, e1n)
    return x_next, xn_next, pre_next


def build_nc(loop_iters=1, debug_outputs=False):
    nc = bacc.Bacc(None, target_bir_lowering=False)
    dram = {
        "x": nc.dram_tensor("x", [C, S], F32R, kind="ExternalInput"),
        "y": nc.dram_tensor("y", [C, S], F32, kind="ExternalOutput"),
        "r_scr": nc.dram_tensor("r_scr", [NH, S], F32),
    }
    WDT_D = FP8 if FP8ALL else F32
    w_in = {
        "wqk": nc.dram_tensor("wqk", [KS, 128, NH, 128], WDT_D, kind="ExternalInput"),
        "wvt": nc.dram_tensor("wvt", [KS, 128, 512], WDT_D, kind="ExternalInput"),
        "wpt": nc.dram_tensor("wpt", [KS, 128, 512], WDT_D, kind="ExternalInput"),
        "bqk": nc.dram_tensor("bqk", [128, NH], F32, kind="ExternalInput"),
        "b_eff": nc.dram_tensor("b_eff", [128, OC], F32, kind="ExternalInput"),
        "gamma": nc.dram_tensor("gamma", [128, OC], F32, kind="ExternalInput"),
        "beta": nc.dram_tensor("beta", [128, OC], F32, kind="ExternalInput"),
        "gn_fwd": nc.dram_tensor("gn_fwd", [OC, 128, 32], F32, kind="ExternalInput"),
        "gn_bwd": nc.dram_tensor("gn_bwd", [OC, 32, 128], F32, kind="ExternalInput"),
        "ident": nc.dram_tensor("ident", [128, 128], F32R, kind="ExternalInput"),
    }

    with tile.TileContext(nc) as tctx:
        with (
            tctx.tile_pool(name="const", bufs=1) as cp,
            tctx.tile_pool(name="sb1", bufs=1) as sb1,
            tctx.tile_pool(name="xp", bufs=2) as xp,
            tctx.tile_pool(name="qk", bufs=NH) as qkp,
            tctx.tile_pool(name="exp", bufs=16) as exp_p,
            tctx.tile_pool(name="a", bufs=NP) as ap_,
            tctx.tile_pool(name="recip", bufs=2) as rp,
            tctx.tile_pool(name="rs", bufs=2) as rsp,
            tctx.tile_pool(name="out", bufs=2) as outp,
            tctx.tile_pool(name="ps_st", bufs=3, space="PSUM") as ps_st,
            tctx.tile_pool(name="ps_av", bufs=2, space="PSUM") as ps_av,
        ):
            pools = dict(sb1=sb1, xp=xp, qk=qkp, exp=exp_p, a=ap_,
                         recip=rp, rs=rsp, out=outp, ps_st=ps_st, ps_av=ps_av)

            # tiny consts first (masks gate the GN combine matmuls)
            consts = {}
            for nm in ("bqk", "b_eff", "gamma", "beta"):
                consts[nm] = cp.tile([128, list(w_in[nm].shape)[1]], F32, name=nm)
                nc.sync.dma_start(consts[nm][:], w_in[nm].ap())
            consts["gn_fwd"] = cp.tile([128, OC, 32], F32, name="gn_fwd")
            nc.sync.dma_start(consts["gn_fwd"][:], w_in["gn_fwd"].ap().rearrange("j p g -> p j g"))
            consts["gn_bwd"] = cp.tile([32, OC, 128], F32, name="gn_bwd")
            nc.sync.dma_start(consts["gn_bwd"][:], w_in["gn_bwd"].ap().rearrange("j g c -> g j c"))
            consts["ident"] = cp.tile([128, 128], F32R, name="ident")
            nc.sync.dma_start(consts["ident"][:], w_in["ident"].ap())
            # packed AV lhsT: [t-part, tsub, head, 128]; even head: [v|1|0*63],
            # odd head: [1|0*63|v]. ones/zero lanes initialized once.
            if AVDR:
                # DoubleRow: [ki, t-pair u, head, ko, m] fp8e4
                consts["av_lhs"] = cp.tile([128, TC // 2, NH, 2, 128], FP8, name="av_lhs")
                av4 = consts["av_lhs"][:].rearrange(
                    "p u (pr tw) ko m -> p u pr tw ko m", tw=2)
                nc.vector.memset(av4[:, :, :, 0, :, 64:65], 1.0)
                nc.vector.memset(av4[:, :, :, 0, :, 65:128], 0.0)
                nc.vector.memset(av4[:, :, :, 1, :, 0:1], 1.0)
                nc.vector.memset(av4[:, :, :, 1, :, 1:64], 0.0)
            else:
                consts["av_lhs"] = cp.tile([128, TC, NH, 128], BF16, name="av_lhs")
                av4 = consts["av_lhs"][:].rearrange("p t (pr tw) m -> p t pr tw m", tw=2)
                nc.vector.memset(av4[:, :, :, 0, 64:65], 1.0)
                nc.vector.memset(av4[:, :, :, 0, 65:128], 0.0)
                nc.vector.memset(av4[:, :, :, 1, 0:1], 1.0)
                nc.vector.memset(av4[:, :, :, 1, 1:64], 0.0)
            consts["eps"] = cp.tile([128, 1], F32, name="eps")
            nc.vector.memset(consts["eps"][:], EPS)
            consts["negc"] = cp.tile([128, 1], F32, name="negc")
            nc.vector.memset(consts["negc"][:], -2.0)

            x_pre = None
            if loop_iters == 1:
                x_pre = emit_x_load(nc, pools, dram)

            wqk_r = w_in["wqk"].ap().rearrange("k p h m -> p k h m")
            WDT = FP8 if FP8ALL else F32R
            consts["wqk"] = cp.tile([128, KS, NH, 128], WDT, name="wqk")
            consts["wvt"] = cp.tile([128, KS, 512], WDT, name="wvt")
            consts["wpt"] = cp.tile([128, KS, 512], WDT, name="wpt")
            for h2 in range(NH):
                nc.gpsimd.dma_start(consts["wqk"][:, :, h2:h2 + 1, :], wqk_r[:, :, h2:h2 + 1, :])
                if h2 == 1:
                    nc.gpsimd.dma_start(consts["wvt"][:], w_in["wvt"].ap().rearrange("k p n -> p k n"))
            nc.gpsimd.dma_start(consts["wpt"][:], w_in["wpt"].ap().rearrange("k p n -> p k n"))

            if loop_iters > 1:
                # unrolled hardware loop with ping-pong x prefetch: each body
                # computes on the tile prefetched by the previous one, and
                # emits the next body's GroupNorm over its own tail. The
                # unroll amortizes the For_i back-edge drain/barrier.
                UNROLL = UNROLL_ENV if loop_iters % UNROLL_ENV == 0 else 2
                assert loop_iters % UNROLL == 0
                xc = emit_x_load(nc, pools, dram)
                with tctx.For_i(0, loop_iters // UNROLL, 1, hint_engines=(mybir.EngineType.PE,)):
                    xnc, prec = None, None
                    for u in range(UNROLL):
                        xc, xnc, prec = emit_body(nc, tctx, pools, dram, consts,
                                                  x_sb=xc, xn_sb=xnc, prefetch=True,
                                                  hoist_gn=(u < UNROLL - 1), pre=prec)
            else:
                emit_body(nc, tctx, pools, dram, consts, x_sb=x_pre, xn_sb=None)

    nc.compile()
    return nc


def prep_weights(gamma, beta, w_qkv, b_qkv, w_proj, b_proj):
    q_rows = np.concatenate([np.arange(192 * h, 192 * h + 64) for h in range(NH)])
    k_rows = q_rows + 64
    v_rows = q_rows + 128
    chunk_rows = []
    for p in range(NH // 2):
        chunk_rows.append(np.concatenate([q_rows[128 * p:128 * p + 64],
                                          q_rows[128 * p + 64:128 * p + 128]]))
        chunk_rows.append(np.concatenate([k_rows[128 * p:128 * p + 64],
                                          k_rows[128 * p + 64:128 * p + 128]]))
    wqk = np.stack([w_qkv[rows, :] for rows in chunk_rows])       # [8, 128, 512]
    tmp = wqk.transpose(2, 0, 1)          # [512(c), 8(h), 128(m)]
    wqk_t = np.ascontiguousarray(tmp.reshape(KS, 128, NH, 128))
    bqk = np.ascontiguousarray(np.stack([b_qkv[rows] for rows in chunk_rows], axis=1))

    wv = w_qkv[v_rows, :]
    wvt = np.ascontiguousarray(wv.T.reshape(KS, 128, 512))
    wpt = np.ascontiguousarray(w_proj.T.reshape(KS, 128, 512))

    b_v = b_qkv[v_rows]
    b_eff = (b_proj.astype(np.float64) + w_proj.astype(np.float64) @ b_v.astype(np.float64)).astype(np.float32)
    b_eff_t = np.ascontiguousarray(b_eff.reshape(OC, 128).T)
    gamma_t = np.ascontiguousarray(np.asarray(gamma, np.float32).reshape(OC, 128).T)
    beta_t = np.ascontiguousarray(np.asarray(beta, np.float32).reshape(OC, 128).T)

    gn_fwd = np.zeros((OC, 128, 32), np.float32)
    gn_bwd = np.zeros((OC, 32, 128), np.float32)
    for j in range(OC):
        for pp in range(128):
            gn_fwd[j, pp, (128 * j + pp) // 16] = 1.0 / 16.0
            gn_bwd[j, (128 * j + pp) // 16, pp] = 1.0
    ident = np.eye(128, dtype=np.float32)
    if FP8ALL:
        f8 = mybir.dt.np(FP8)
        wqk_t = wqk_t.astype(f8)
        wvt = wvt.astype(f8)
        wpt = np.ascontiguousarray(wpt * 64.0).astype(f8)
    return {"wqk": wqk_t, "wvt": wvt, "wpt": wpt, "bqk": bqk, "b_eff": b_eff_t,
            "gamma": gamma_t, "beta": beta_t, "gn_fwd": gn_fwd, "gn_bwd": gn_bwd,
            "ident": ident}


_STATE = {}
N_CORES = 8


class _SpmdRunner:
    def __init__(self, nc, n_cores):
        import jax
        from jax.sharding import Mesh, PartitionSpec
        from jax.experimental.shard_map import shard_map
        from concourse.bass2jax import _bass_exec_p, partition_id_tensor, install_neuronx_cc_hook
        install_neuronx_cc_hook()
        self.n_cores = n_cores
        partition_name = nc.partition_id_tensor.name if nc.partition_id_tensor else None
        in_names, out_names, out_avals, zero_outs = [], [], [], []
        for alloc in nc.m.functions[0].allocations:
            if not isinstance(alloc, mybir.MemoryLocationSet):
                continue
            name = alloc.memorylocations[0].name
            if alloc.kind == "ExternalInput":
                if name != partition_name:
                    in_names.append(name)
            elif alloc.kind == "ExternalOutput":
                out_names.append(name)
                shape = tuple(alloc.tensor_shape)
                dtype = mybir.dt.np(alloc.dtype)
                out_avals.append(jax.core.ShapedArray(shape, dtype))
                zero_outs.append(np.zeros(shape, dtype))
        self.in_names, self.out_names = in_names, out_names
        self.out_avals, self.zero_outs = out_avals, zero_outs
        n_params, n_outs = len(in_names), len(out_avals)
        all_in_names = list(in_names) + list(out_names)
        if partition_name is not None:
            all_in_names.append(partition_name)

        def _body(*args):
            operands = list(args)
            if partition_name is not None:
                operands.append(partition_id_tensor())
            outs = _bass_exec_p.bind(
                *operands, out_avals=tuple(out_avals), in_names=tuple(all_in_names),
                out_names=tuple(out_names), lowering_input_output_aliases=(),
                sim_require_finite=True, sim_require_nnan=True, nc=nc)
            return tuple(outs)

        devices = jax.devices()[:n_cores]
        mesh = Mesh(np.asarray(devices), ("core",))
        in_specs = (PartitionSpec("core"),) * (n_params + n_outs)
        out_specs = (PartitionSpec("core"),) * n_outs
        self.sharded = jax.jit(
            shard_map(_body, mesh=mesh, in_specs=in_specs, out_specs=out_specs, check_rep=False),
            donate_argnums=tuple(range(n_params, n_params + n_outs)), keep_unused=True)

    def __call__(self, in_maps):
        n_cores = self.n_cores
        per_core = [[np.asarray(m[name]) for name in self.in_names] for m in in_maps]
        concat_in = [np.concatenate([per_core[c][i] for c in range(n_cores)], axis=0)
                     for i in range(len(self.in_names))]
        concat_zeros = [np.zeros((n_cores * z.shape[0], *z.shape[1:]), z.dtype)
                        for z in self.zero_outs]
        out_arrs = self.sharded(*concat_in, *concat_zeros)
        return [
            {name: np.asarray(out_arrs[i]).reshape(n_cores, *self.out_avals[i].shape)[c]
             for i, name in enumerate(self.out_names)}
            for c in range(n_cores)
        ]


def kernel(x, gamma, beta, w_qkv, b_qkv, w_proj, b_proj):
    x = np.asarray(x, np.float32)
    assert x.shape == (8, C, 32, 32), x.shape
    w = prep_weights(np.asarray(gamma, np.float32), np.asarray(beta, np.float32),
                     np.asarray(w_qkv, np.float32), np.asarray(b_qkv, np.float32),
                     np.asarray(w_proj, np.float32), np.asarray(b_proj, np.float32))
    if "runner" not in _STATE:
        nc = build_nc(loop_iters=1)
        _STATE["runner"] = _SpmdRunner(nc, N_CORES)
    in_maps = []
    for b in range(N_CORES):
        m = {"x": np.ascontiguousarray(x[b].reshape(C, S))}
        m.update(w)
        in_maps.append(m)
    res = _STATE["runner"](in_maps)
    out = np.stack([res[b]["y"] for b in range(N_CORES)]).reshape(8, C, 32, 32)
    return out.astype(np.float32)
